# revision 2
# baseline (speedup 1.0000x reference)
"""AttentionClustering (vq_codebook) Trainium2 kernel, 8-core data parallel.

Shard: 8 cores = 4 images x 2 half-images (128 output rows each). Odd cores
get a vertically flipped shard + row-flipped conv weights so every core's
program is identical (true image edge at local top, interior halo at bottom).

Math: q1 = relu(conv3x3(x, w1) + b1); q2 = relu(conv3x3(q1, w2) + b2)  (both
with replicate padding); then the 1x1 conv + cluster-distance softmax folds to
  logit[px, k] = sum_ci q2[ci, px] * muW[k, ci] + bp[k]
  muW = 2 * mu @ W3,  bp = 2 * mu @ b3 - |mu|^2      (|q|^2 cancels in softmax)
  out[px] = sum_k softmax_k(logit) * label[k]

All matmuls run as float32r (12-bit mantissa, full PE rate at N>=256).
"""
import sys
if '/opt/trn_rl_repo' not in sys.path:
    sys.path.insert(0, '/opt/trn_rl_repo')

import numpy as np
import concourse.bass as bass
import concourse.mybir as mybir
from concourse import bacc, tile
from concourse.bass_utils import run_bass_kernel_spmd

F32 = mybir.dt.float32
F32R = mybir.dt.float32r
AF = mybir.ActivationFunctionType
ALU = mybir.AluOpType
AX = mybir.AxisListType

B, CIN, H, W = 4, 64, 256, 256
Q, K = 256, 16
RB = 16           # output rows per band
NBAND = 8         # bands per core (128 rows)
NCORES = 8

_cached = {}


def round_fp32r(x):
    u = np.ascontiguousarray(x, np.float32).view(np.uint32)
    lsb = (u >> 12) & 1
    return ((u + 0x7FF + lsb) & 0xFFFFF000).astype(np.uint32).view(np.float32)


def build_nc():
    nc = bacc.Bacc("TRN2", target_bir_lowering=False, debug=False)

    xh = nc.declare_dram_parameter("xh", [CIN, 132, W + 2], F32R, isOutput=False)
    w1a = nc.declare_dram_parameter("w1a", [2, 3, 128, 128], F32R, isOutput=False)
    w1r = nc.declare_dram_parameter("w1r", [2, 128, 128], F32R, isOutput=False)
    w1s = nc.declare_dram_parameter("w1s", [2, 64, 128], F32R, isOutput=False)
    w2l = nc.declare_dram_parameter("w2l", [36, 128, 128], F32R, isOutput=False)
    muw = nc.declare_dram_parameter("muw", [2, 128, K], F32R, isOutput=False)
    bp = nc.declare_dram_parameter("bp", [128, K], F32, isOutput=False)
    lab = nc.declare_dram_parameter("lab", [128, K], F32, isOutput=False)
    b1 = nc.declare_dram_parameter("b1", [2, 128, 1], F32, isOutput=False)
    b2 = nc.declare_dram_parameter("b2", [2, 128, 1], F32, isOutput=False)
    outd = nc.declare_dram_parameter("out", [128, W], F32, isOutput=True)

    with tile.TileContext(nc) as tc:
        with tc.tile_pool(name="singles", bufs=1) as singles, \
             tc.tile_pool(name="xpool", bufs=2) as xpool, \
             tc.tile_pool(name="q1pool", bufs=1) as q1pool, \
             tc.tile_pool(name="q2pool", bufs=2) as q2pool, \
             tc.tile_pool(name="smx", bufs=2) as smx, \
             tc.tile_pool(name="obuf", bufs=2) as obuf, \
             tc.tile_pool(name="ps1", bufs=3, space="PSUM") as ps1, \
             tc.tile_pool(name="ps2", bufs=2, space="PSUM") as ps2, \
             tc.tile_pool(name="psl", bufs=2, space="PSUM") as psl:

            # ---- resident weights -------------------------------------
            w1a_sb = {}
            for mc in range(2):
                for dr in range(3):
                    t = singles.tile([128, 128], F32R, tag=f"w1a{mc}{dr}", name=f"w1a{mc}{dr}")
                    nc.sync.dma_start(out=t, in_=w1a.ap()[mc, dr])
                    w1a_sb[mc, dr] = t
            w1r_sb = {}
            w1s_sb = {}
            for mc in range(2):
                t = singles.tile([128, 128], F32R, tag=f"w1r{mc}", name=f"w1r{mc}")
                nc.sync.dma_start(out=t, in_=w1r.ap()[mc])
                w1r_sb[mc] = t
                t = singles.tile([64, 128], F32R, tag=f"w1s{mc}", name=f"w1s{mc}")
                nc.sync.dma_start(out=t, in_=w1s.ap()[mc])
                w1s_sb[mc] = t
            w2_sb = {}
            for kc in range(2):
                for ti in range(9):
                    for mc in range(2):
                        idx = (kc * 9 + ti) * 2 + mc
                        t = singles.tile([128, 128], F32R, tag=f"w2_{idx}", name=f"w2_{idx}")
                        nc.sync.dma_start(out=t, in_=w2l.ap()[idx])
                        w2_sb[kc, ti, mc] = t
            muw_sb = {}
            for kc in range(2):
                t = singles.tile([128, K], F32R, tag=f"muw{kc}", name=f"muw{kc}")
                nc.sync.dma_start(out=t, in_=muw.ap()[kc])
                muw_sb[kc] = t
            bp_sb = singles.tile([128, K], F32, tag="bp")
            nc.sync.dma_start(out=bp_sb, in_=bp.ap())
            lab_sb = singles.tile([128, K], F32, tag="lab")
            nc.sync.dma_start(out=lab_sb, in_=lab.ap())
            b1_sb = {}
            b2_sb = {}
            for mc in range(2):
                t = singles.tile([128, 1], F32, tag=f"b1{mc}", name=f"b1{mc}")
                nc.sync.dma_start(out=t, in_=b1.ap()[mc])
                b1_sb[mc] = t
                t = singles.tile([128, 1], F32, tag=f"b2{mc}", name=f"b2{mc}")
                nc.sync.dma_start(out=t, in_=b2.ap()[mc])
                b2_sb[mc] = t

            # ---- bands ------------------------------------------------
            for band in range(NBAND):
                r0 = RB * band
                # x halo in two packings:
                #  xa: p0-63 = xh rows r0..r0+19, p64-127 = same shifted +1 col
                #  xr: p0-63 = xh rows,           p64-127 = same shifted +1 row
                xa = xpool.tile([128, 20, W + 2], F32R, tag="xa", name="xa")
                nc.sync.dma_start(out=xa[0:64], in_=xh.ap()[:, r0:r0 + 20, :])
                nc.sync.dma_start(out=xa[64:128, :, 0:W + 1],
                                  in_=xh.ap()[:, r0:r0 + 20, 1:W + 2])
                xr = xpool.tile([128, 20, W + 2], F32R, tag="xr", name="xr")
                nc.sync.dma_start(out=xr[0:64], in_=xh.ap()[:, r0:r0 + 20, :])
                nc.sync.dma_start(out=xr[64:128, 0:19, :],
                                  in_=xh.ap()[:, r0 + 1:r0 + 20, :])

                # q1 band buffer: slot j = q1 row (r0 - 1 + j), cols 1..256
                # real, cols 0/257 replicate pads.
                q1b = {}
                for mc in range(2):
                    q1b[mc] = q1pool.tile([128, RB + 2, W + 2], F32R, tag=f"q1_{mc}", name=f"q1_{mc}")

                # conv1: q1 slot j needs xh local rows j+dr (pairs), and
                # taps (0,2),(1,2) from xr row j, tap (2,2) from xa row j+2.
                if band == 0:
                    groups1 = [(1, 2), (3, 2), (5, 2), (7, 2), (9, 2),
                               (11, 2), (13, 2), (15, 2), (17, 1)]
                else:
                    groups1 = [(j, 2) for j in range(0, RB + 2, 2)]
                for j, nr in groups1:
                    for mc in range(2):
                        ps = ps1.tile([128, nr, W], F32, tag="c1ps", name="c1ps")
                        for dr in range(3):
                            nc.tensor.matmul(
                                ps, w1a_sb[mc, dr],
                                xa[:, j + dr:j + dr + nr, 0:W],
                                start=(dr == 0), stop=False)
                        nc.tensor.matmul(ps, w1r_sb[mc],
                                         xr[:, j:j + nr, 2:W + 2],
                                         start=False, stop=False)
                        nc.tensor.matmul(ps, w1s_sb[mc],
                                         xa[0:64, j + 2:j + 2 + nr, 2:W + 2],
                                         start=False, stop=True)
                        nc.scalar.activation(
                            out=q1b[mc][:, j:j + nr, 1:W + 1], in_=ps,
                            func=AF.Relu, bias=b1_sb[mc], scale=1.0)
                # replicate pads: cols, then (band 0) top row
                for mc in range(2):
                    lo = 1 if band == 0 else 0
                    nc.vector.tensor_copy(
                        out=q1b[mc][:, lo:RB + 2, 0:1],
                        in_=q1b[mc][:, lo:RB + 2, 1:2])
                    nc.vector.tensor_copy(
                        out=q1b[mc][:, lo:RB + 2, W + 1:W + 2],
                        in_=q1b[mc][:, lo:RB + 2, W:W + 1])
                    if band == 0:
                        nc.vector.tensor_copy(
                            out=q1b[mc][:, 0:1, :], in_=q1b[mc][:, 1:2, :])

                ob = obuf.tile([128, RB // 2, 4], F32, tag="ob", name="ob")
                for g in range(RB // 2):
                    # conv2 -> q2 (2 output rows x 256 cols per group)
                    q2t = {}
                    for mc in range(2):
                        ps = ps2.tile([128, 2, W], F32, tag="c2ps", name="c2ps")
                        n_mm = 0
                        for kc in range(2):
                            for dr in range(3):
                                for dc in range(3):
                                    nc.tensor.matmul(
                                        ps, w2_sb[kc, dr * 3 + dc, mc],
                                        q1b[kc][:, 2 * g + dr:2 * g + dr + 2,
                                                dc:dc + W],
                                        start=(n_mm == 0), stop=(n_mm == 17))
                                    n_mm += 1
                        q2t[mc] = q2pool.tile([128, 2, W], F32R, tag=f"q2_{mc}", name=f"q2_{mc}")
                        nc.scalar.activation(out=q2t[mc], in_=ps, func=AF.Relu,
                                             bias=b2_sb[mc], scale=1.0)
                    # logits: [128 px, K] per 128-px slice, q2 stationary
                    pl = psl.tile([128, 4, K], F32, tag="lps", name="lps")
                    for j in range(4):
                        for kc in range(2):
                            q2flat = q2t[kc].rearrange("p a b -> p (a b)")
                            nc.tensor.matmul(
                                pl[:, j, :], q2flat[:, 128 * j:128 * (j + 1)],
                                muw_sb[kc], start=(kc == 0), stop=(kc == 1))
                    # softmax over K (free axis) + label contraction
                    li = smx.tile([128, 4, K], F32, tag="li", name="li")
                    nc.vector.tensor_tensor(
                        li, pl,
                        bp_sb.unsqueeze(1).to_broadcast([128, 4, K]),
                        ALU.add)
                    mx = smx.tile([128, 4], F32, tag="mx", name="mx")
                    nc.vector.reduce_max(mx, li, axis=AX.X)
                    ls = smx.tile([128, 4, K], F32, tag="ls", name="ls")
                    nc.vector.tensor_tensor(
                        ls, li,
                        mx.unsqueeze(2).to_broadcast([128, 4, K]),
                        ALU.subtract)
                    ex = smx.tile([128, 4, K], F32, tag="ex", name="ex")
                    nc.scalar.activation(out=ex, in_=ls, func=AF.Exp)
                    el = smx.tile([128, 4, K], F32, tag="el", name="el")
                    nc.vector.tensor_tensor(
                        el, ex,
                        lab_sb.unsqueeze(1).to_broadcast([128, 4, K]),
                        ALU.mult)
                    ssum = smx.tile([128, 4], F32, tag="ssum", name="ssum")
                    nc.vector.reduce_sum(ssum, ex, axis=AX.X)
                    wsum = smx.tile([128, 4], F32, tag="wsum", name="wsum")
                    nc.vector.reduce_sum(wsum, el, axis=AX.X)
                    rs = smx.tile([128, 4], F32, tag="rs", name="rs")
                    nc.vector.reciprocal(rs, ssum)
                    nc.vector.tensor_tensor(ob[:, g], wsum, rs, ALU.mult)

                # out[p, g, r, jj] -> dram row r0+2g+r, col 128*jj + p
                nc.sync.dma_start(
                    out=outd.ap()[r0:r0 + RB, :].rearrange(
                        "(g r) (jj p) -> p g r jj", r=2, p=128),
                    in_=ob.rearrange("p g (r jj) -> p g r jj", r=2))

    nc.compile()
    return nc


def prep_inputs(x, w1, b1, w2, b2, w3, b3, mu, label):
    """Full inputs -> per-core in_maps."""
    w3m = w3[:, :, 0, 0]
    muW = 2.0 * (mu @ w3m)                                   # [K, Q]
    bpv = (2.0 * (mu @ b3) - (mu * mu).sum(1)).astype(np.float32)

    def pack_w(w1f, w2f):
        cinw = w1f.shape[1]
        w1a = np.empty((2, 3, 128, 128), np.float32)
        w1r = np.empty((2, 128, 128), np.float32)
        w1s = np.empty((2, 64, 128), np.float32)
        for mc in range(2):
            ms = slice(128 * mc, 128 * (mc + 1))
            for dr in range(3):
                w1a[mc, dr, 0:64] = w1f[ms, :, dr, 0].T
                w1a[mc, dr, 64:128] = w1f[ms, :, dr, 1].T
            w1r[mc, 0:64] = w1f[ms, :, 0, 2].T
            w1r[mc, 64:128] = w1f[ms, :, 1, 2].T
            w1s[mc] = w1f[ms, :, 2, 2].T
        w2p = np.empty((36, 128, 128), np.float32)
        for kc in range(2):
            for dr in range(3):
                for dc in range(3):
                    for mc in range(2):
                        idx = (kc * 9 + dr * 3 + dc) * 2 + mc
                        w2p[idx] = w2f[128 * mc:128 * (mc + 1),
                                       128 * kc:128 * (kc + 1), dr, dc].T
        return round_fp32r(w1a), round_fp32r(w1r), round_fp32r(w1s), round_fp32r(w2p)

    packs = {}
    packs[0] = pack_w(w1, w2)
    packs[1] = pack_w(w1[:, :, ::-1, :], w2[:, :, ::-1, :])

    muwp = np.empty((2, 128, K), np.float32)
    for kc in range(2):
        muwp[kc] = muW[:, 128 * kc:128 * (kc + 1)].T
    muwp = round_fp32r(muwp)
    bpt = np.broadcast_to(bpv[None, :], (128, K)).copy()
    labt = np.broadcast_to(label[None, :].astype(np.float32), (128, K)).copy()
    b1t = np.empty((2, 128, 1), np.float32)
    b2t = np.empty((2, 128, 1), np.float32)
    for mc in range(2):
        b1t[mc, :, 0] = b1[128 * mc:128 * (mc + 1)]
        b2t[mc, :, 0] = b2[128 * mc:128 * (mc + 1)]

    rows = np.clip(np.arange(132) - 2, 0, H - 1)
    cols = np.clip(np.arange(W + 2) - 1, 0, W - 1)
    in_maps = []
    for core in range(NCORES):
        img, half = core // 2, core % 2
        xl = x[img] if half == 0 else x[img, :, ::-1, :]
        xhv = round_fp32r(np.ascontiguousarray(xl[:, rows][:, :, cols]))
        w1ap, w1rp, w1sp, w2p = packs[half]
        in_maps.append({
            'xh': xhv, 'w1a': w1ap, 'w1r': w1rp, 'w1s': w1sp, 'w2l': w2p,
            'muw': muwp, 'bp': bpt, 'lab': labt, 'b1': b1t, 'b2': b2t,
        })
    return in_maps


def gather(results, dtype=np.float32):
    out = np.empty((B, 1, H, W), dtype)
    for core in range(NCORES):
        img, half = core // 2, core % 2
        o = results[core]['out']
        if half == 0:
            out[img, 0, 0:128] = o
        else:
            out[img, 0, 128:256] = o[::-1]
    return out


def get_nc():
    if 'nc' not in _cached:
        _cached['nc'] = build_nc()
    return _cached['nc']


def kernel(x, w1, b1, w2, b2, w3, b3, mu, label, **run_kwargs):
    nc = get_nc()
    in_maps = prep_inputs(
        np.asarray(x, np.float32), np.asarray(w1, np.float32),
        np.asarray(b1, np.float32), np.asarray(w2, np.float32),
        np.asarray(b2, np.float32), np.asarray(w3, np.float32),
        np.asarray(b3, np.float32), np.asarray(mu, np.float32),
        np.asarray(label, np.float32))
    res = run_bass_kernel_spmd(nc, in_maps, core_ids=list(range(NCORES)),
                               **run_kwargs)
    out = gather(res.results)
    if run_kwargs:
        _cached['last_result'] = res
    return out


# revision 5
# speedup vs baseline: 1.4010x; 1.4010x over previous
"""AttentionClustering (vq_codebook) Trainium2 kernel, 8-core data parallel.

Shard: 8 cores = 4 images x 2 half-images (128 output rows each). Odd cores
get a vertically flipped shard + row-flipped conv weights so every core's
program is identical (true image edge at local top, interior halo at bottom).

Math: q1 = relu(conv3x3(x, w1) + b1); q2 = relu(conv3x3(q1, w2) + b2)  (both
with replicate padding); then the 1x1 conv + cluster-distance softmax folds to
  logit[px, k] = sum_ci q2[ci, px] * muW[k, ci] + bp[k]
  muW = 2 * mu @ W3,  bp = 2 * mu @ b3 - |mu|^2      (|q|^2 cancels in softmax)
  out[px] = sum_k softmax_k(logit) * label[k]

All matmuls run as float32r (12-bit mantissa, full PE rate at N>=256).
"""
import sys
if '/opt/trn_rl_repo' not in sys.path:
    sys.path.insert(0, '/opt/trn_rl_repo')

import numpy as np
import concourse.bass as bass
import concourse.mybir as mybir
from concourse import bacc, tile
from concourse.bass_utils import run_bass_kernel_spmd

F32 = mybir.dt.float32
F32R = mybir.dt.float32r
F16 = mybir.dt.float16
AF = mybir.ActivationFunctionType
ALU = mybir.AluOpType
AX = mybir.AxisListType

B, CIN, H, W = 4, 64, 256, 256
Q, K = 256, 16
RB = 16           # output rows per band
NBAND = 8         # bands per core (128 rows)
NCORES = 8

_cached = {}


def round_fp32r(x):
    u = np.ascontiguousarray(x, np.float32).view(np.uint32)
    lsb = (u >> 12) & 1
    return ((u + 0x7FF + lsb) & 0xFFFFF000).astype(np.uint32).view(np.float32)


def build_nc():
    nc = bacc.Bacc("TRN2", target_bir_lowering=False, debug=False)

    xh = nc.declare_dram_parameter("xh", [CIN, 132, W + 2], F16, isOutput=False)
    w1a = nc.declare_dram_parameter("w1a", [2, 3, 128, 128], F16, isOutput=False)
    w1r = nc.declare_dram_parameter("w1r", [2, 128, 128], F16, isOutput=False)
    w1s = nc.declare_dram_parameter("w1s", [2, 64, 128], F16, isOutput=False)
    w2l = nc.declare_dram_parameter("w2l", [36, 128, 128], F16, isOutput=False)
    muw = nc.declare_dram_parameter("muw", [2, 128, K], F16, isOutput=False)
    bp = nc.declare_dram_parameter("bp", [128, K], F32, isOutput=False)
    lab = nc.declare_dram_parameter("lab", [128, K], F32, isOutput=False)
    b1 = nc.declare_dram_parameter("b1", [2, 128, 1], F32, isOutput=False)
    b2 = nc.declare_dram_parameter("b2", [2, 128, 1], F32, isOutput=False)
    outd = nc.declare_dram_parameter("out", [128, W], F32, isOutput=True)

    with tile.TileContext(nc) as tc:
        with tc.tile_pool(name="singles", bufs=1) as singles, \
             tc.tile_pool(name="xpool", bufs=2) as xpool, \
             tc.tile_pool(name="q1pool", bufs=1) as q1pool, \
             tc.tile_pool(name="q2pool", bufs=2) as q2pool, \
             tc.tile_pool(name="smx", bufs=2) as smx, \
             tc.tile_pool(name="obuf", bufs=2) as obuf, \
             tc.tile_pool(name="ps1", bufs=3, space="PSUM") as ps1, \
             tc.tile_pool(name="ps2", bufs=2, space="PSUM") as ps2, \
             tc.tile_pool(name="psl", bufs=2, space="PSUM") as psl:

            # ---- resident weights -------------------------------------
            w1a_sb = {}
            for mc in range(2):
                for dr in range(3):
                    t = singles.tile([128, 128], F16, tag=f"w1a{mc}{dr}", name=f"w1a{mc}{dr}")
                    nc.sync.dma_start(out=t, in_=w1a.ap()[mc, dr])
                    w1a_sb[mc, dr] = t
            w1r_sb = {}
            w1s_sb = {}
            for mc in range(2):
                t = singles.tile([128, 128], F16, tag=f"w1r{mc}", name=f"w1r{mc}")
                nc.sync.dma_start(out=t, in_=w1r.ap()[mc])
                w1r_sb[mc] = t
                t = singles.tile([64, 128], F16, tag=f"w1s{mc}", name=f"w1s{mc}")
                nc.sync.dma_start(out=t, in_=w1s.ap()[mc])
                w1s_sb[mc] = t
            # band-0 x halo first so conv1 can start before w2 finishes loading
            def load_xband(r0):
                xa = xpool.tile([128, 20, W + 2], F16, tag="xa", name="xa")
                nc.sync.dma_start(out=xa[0:64], in_=xh.ap()[:, r0:r0 + 20, :])
                nc.sync.dma_start(out=xa[64:128, :, 0:W + 1],
                                  in_=xh.ap()[:, r0:r0 + 20, 1:W + 2])
                xr = xpool.tile([128, 20, W + 2], F16, tag="xr", name="xr")
                nc.sync.dma_start(out=xr[0:64], in_=xh.ap()[:, r0:r0 + 20, :])
                nc.sync.dma_start(out=xr[64:128, 0:19, :],
                                  in_=xh.ap()[:, r0 + 1:r0 + 20, :])
                return xa, xr

            xband0 = load_xband(0)

            w2_sb = {}
            for kc in range(2):
                for ti in range(9):
                    for mc in range(2):
                        idx = (kc * 9 + ti) * 2 + mc
                        t = singles.tile([128, 128], F16, tag=f"w2_{idx}", name=f"w2_{idx}")
                        nc.sync.dma_start(out=t, in_=w2l.ap()[idx])
                        w2_sb[kc, ti, mc] = t
            muw_sb = {}
            for kc in range(2):
                t = singles.tile([128, K], F16, tag=f"muw{kc}", name=f"muw{kc}")
                nc.sync.dma_start(out=t, in_=muw.ap()[kc])
                muw_sb[kc] = t
            bp_sb = singles.tile([128, K], F32, tag="bp")
            nc.sync.dma_start(out=bp_sb, in_=bp.ap())
            lab_sb = singles.tile([128, K], F32, tag="lab")
            nc.sync.dma_start(out=lab_sb, in_=lab.ap())
            b1_sb = {}
            b2_sb = {}
            for mc in range(2):
                t = singles.tile([128, 1], F32, tag=f"b1{mc}", name=f"b1{mc}")
                nc.sync.dma_start(out=t, in_=b1.ap()[mc])
                b1_sb[mc] = t
                t = singles.tile([128, 1], F32, tag=f"b2{mc}", name=f"b2{mc}")
                nc.sync.dma_start(out=t, in_=b2.ap()[mc])
                b2_sb[mc] = t

            # ---- bands ------------------------------------------------
            for band in range(NBAND):
                r0 = RB * band
                # x halo in two packings:
                #  xa: p0-63 = xh rows r0..r0+19, p64-127 = same shifted +1 col
                #  xr: p0-63 = xh rows,           p64-127 = same shifted +1 row
                xa, xr = xband0 if band == 0 else load_xband(r0)

                # q1 band buffer: slot j = q1 row (r0 - 1 + j), cols 1..256
                # real, cols 0/257 replicate pads.
                q1b = {}
                for mc in range(2):
                    q1b[mc] = q1pool.tile([128, RB + 2, W + 2], F16, tag=f"q1_{mc}", name=f"q1_{mc}")

                # conv1: q1 slot j needs xh local rows j+dr (pairs), and
                # taps (0,2),(1,2) from xr row j, tap (2,2) from xa row j+2.
                if band == 0:
                    groups1 = [(1, 2), (3, 2), (5, 2), (7, 2), (9, 2),
                               (11, 2), (13, 2), (15, 2), (17, 1)]
                else:
                    groups1 = [(j, 2) for j in range(0, RB + 2, 2)]
                for j, nr in groups1:
                    for mc in range(2):
                        ps = ps1.tile([128, nr, W], F32, tag="c1ps", name="c1ps")
                        for dr in range(3):
                            nc.tensor.matmul(
                                ps, w1a_sb[mc, dr],
                                xa[:, j + dr:j + dr + nr, 0:W],
                                start=(dr == 0), stop=False)
                        nc.tensor.matmul(ps, w1r_sb[mc],
                                         xr[:, j:j + nr, 2:W + 2],
                                         start=False, stop=False)
                        nc.tensor.matmul(ps, w1s_sb[mc],
                                         xa[0:64, j + 2:j + 2 + nr, 2:W + 2],
                                         start=False, stop=True)
                        nc.scalar.activation(
                            out=q1b[mc][:, j:j + nr, 1:W + 1], in_=ps,
                            func=AF.Relu, bias=b1_sb[mc], scale=1.0)
                # replicate pads: cols, then (band 0) top row
                for mc in range(2):
                    lo = 1 if band == 0 else 0
                    nc.vector.tensor_copy(
                        out=q1b[mc][:, lo:RB + 2, 0:1],
                        in_=q1b[mc][:, lo:RB + 2, 1:2])
                    nc.vector.tensor_copy(
                        out=q1b[mc][:, lo:RB + 2, W + 1:W + 2],
                        in_=q1b[mc][:, lo:RB + 2, W:W + 1])
                    if band == 0:
                        nc.vector.tensor_copy(
                            out=q1b[mc][:, 0:1, :], in_=q1b[mc][:, 1:2, :])

                ob = obuf.tile([128, RB // 2, 4], F32, tag="ob", name="ob")
                for g in range(RB // 2):
                    # conv2 -> q2 (2 output rows x 256 cols per group)
                    q2t = {}
                    for mc in range(2):
                        ps = ps2.tile([128, 2, W], F32, tag="c2ps", name="c2ps")
                        n_mm = 0
                        for kc in range(2):
                            for dr in range(3):
                                for dc in range(3):
                                    nc.tensor.matmul(
                                        ps, w2_sb[kc, dr * 3 + dc, mc],
                                        q1b[kc][:, 2 * g + dr:2 * g + dr + 2,
                                                dc:dc + W],
                                        start=(n_mm == 0), stop=(n_mm == 17))
                                    n_mm += 1
                        q2t[mc] = q2pool.tile([128, 2, W], F16, tag=f"q2_{mc}", name=f"q2_{mc}")
                        nc.scalar.activation(out=q2t[mc], in_=ps, func=AF.Relu,
                                             bias=b2_sb[mc], scale=1.0)
                    # logits: [128 px, K] per 128-px slice, q2 stationary
                    pl = psl.tile([128, 4, K], F32, tag="lps", name="lps")
                    for j in range(4):
                        for kc in range(2):
                            q2flat = q2t[kc].rearrange("p a b -> p (a b)")
                            nc.tensor.matmul(
                                pl[:, j, :], q2flat[:, 128 * j:128 * (j + 1)],
                                muw_sb[kc], start=(kc == 0), stop=(kc == 1))
                    # softmax over K (free axis) + label contraction
                    li = smx.tile([128, 4, K], F32, tag="li", name="li")
                    nc.vector.tensor_tensor(
                        li, pl,
                        bp_sb.unsqueeze(1).to_broadcast([128, 4, K]),
                        ALU.add)
                    mx = smx.tile([128, 4], F32, tag="mx", name="mx")
                    nc.vector.reduce_max(mx, li, axis=AX.X)
                    ls = smx.tile([128, 4, K], F32, tag="ls", name="ls")
                    nc.vector.tensor_tensor(
                        ls, li,
                        mx.unsqueeze(2).to_broadcast([128, 4, K]),
                        ALU.subtract)
                    ex = smx.tile([128, 4, K], F32, tag="ex", name="ex")
                    nc.scalar.activation(out=ex, in_=ls, func=AF.Exp)
                    el = smx.tile([128, 4, K], F32, tag="el", name="el")
                    nc.vector.tensor_tensor(
                        el, ex,
                        lab_sb.unsqueeze(1).to_broadcast([128, 4, K]),
                        ALU.mult)
                    ssum = smx.tile([128, 4], F32, tag="ssum", name="ssum")
                    nc.vector.reduce_sum(ssum, ex, axis=AX.X)
                    wsum = smx.tile([128, 4], F32, tag="wsum", name="wsum")
                    nc.vector.reduce_sum(wsum, el, axis=AX.X)
                    rs = smx.tile([128, 4], F32, tag="rs", name="rs")
                    nc.vector.reciprocal(rs, ssum)
                    nc.vector.tensor_tensor(ob[:, g], wsum, rs, ALU.mult)

                # out[p, g, r, jj] -> dram row r0+2g+r, col 128*jj + p
                nc.sync.dma_start(
                    out=outd.ap()[r0:r0 + RB, :].rearrange(
                        "(g r) (jj p) -> p g r jj", r=2, p=128),
                    in_=ob.rearrange("p g (r jj) -> p g r jj", r=2))

    nc.compile()
    return nc


def prep_inputs(x, w1, b1, w2, b2, w3, b3, mu, label):
    """Full inputs -> per-core in_maps."""
    w3m = w3[:, :, 0, 0]
    muW = 2.0 * (mu @ w3m)                                   # [K, Q]
    bpv = (2.0 * (mu @ b3) - (mu * mu).sum(1)).astype(np.float32)

    def pack_w(w1f, w2f):
        cinw = w1f.shape[1]
        w1a = np.empty((2, 3, 128, 128), np.float32)
        w1r = np.empty((2, 128, 128), np.float32)
        w1s = np.empty((2, 64, 128), np.float32)
        for mc in range(2):
            ms = slice(128 * mc, 128 * (mc + 1))
            for dr in range(3):
                w1a[mc, dr, 0:64] = w1f[ms, :, dr, 0].T
                w1a[mc, dr, 64:128] = w1f[ms, :, dr, 1].T
            w1r[mc, 0:64] = w1f[ms, :, 0, 2].T
            w1r[mc, 64:128] = w1f[ms, :, 1, 2].T
            w1s[mc] = w1f[ms, :, 2, 2].T
        w2p = np.empty((36, 128, 128), np.float32)
        for kc in range(2):
            for dr in range(3):
                for dc in range(3):
                    for mc in range(2):
                        idx = (kc * 9 + dr * 3 + dc) * 2 + mc
                        w2p[idx] = w2f[128 * mc:128 * (mc + 1),
                                       128 * kc:128 * (kc + 1), dr, dc].T
        return (w1a.astype(np.float16), w1r.astype(np.float16),
                w1s.astype(np.float16), w2p.astype(np.float16))

    packs = {}
    packs[0] = pack_w(w1, w2)
    packs[1] = pack_w(w1[:, :, ::-1, :], w2[:, :, ::-1, :])

    muwp = np.empty((2, 128, K), np.float32)
    for kc in range(2):
        muwp[kc] = muW[:, 128 * kc:128 * (kc + 1)].T
    muwp = muwp.astype(np.float16)
    bpt = np.broadcast_to(bpv[None, :], (128, K)).copy()
    labt = np.broadcast_to(label[None, :].astype(np.float32), (128, K)).copy()
    b1t = np.empty((2, 128, 1), np.float32)
    b2t = np.empty((2, 128, 1), np.float32)
    for mc in range(2):
        b1t[mc, :, 0] = b1[128 * mc:128 * (mc + 1)]
        b2t[mc, :, 0] = b2[128 * mc:128 * (mc + 1)]

    rows = np.clip(np.arange(132) - 2, 0, H - 1)
    cols = np.clip(np.arange(W + 2) - 1, 0, W - 1)
    in_maps = []
    for core in range(NCORES):
        img, half = core // 2, core % 2
        xl = x[img] if half == 0 else x[img, :, ::-1, :]
        xhv = np.ascontiguousarray(xl[:, rows][:, :, cols]).astype(np.float16)
        w1ap, w1rp, w1sp, w2p = packs[half]
        in_maps.append({
            'xh': xhv, 'w1a': w1ap, 'w1r': w1rp, 'w1s': w1sp, 'w2l': w2p,
            'muw': muwp, 'bp': bpt, 'lab': labt, 'b1': b1t, 'b2': b2t,
        })
    return in_maps


def gather(results, dtype=np.float32):
    out = np.empty((B, 1, H, W), dtype)
    for core in range(NCORES):
        img, half = core // 2, core % 2
        o = results[core]['out']
        if half == 0:
            out[img, 0, 0:128] = o
        else:
            out[img, 0, 128:256] = o[::-1]
    return out


def get_nc():
    if 'nc' not in _cached:
        _cached['nc'] = build_nc()
    return _cached['nc']


def kernel(x, w1, b1, w2, b2, w3, b3, mu, label, **run_kwargs):
    nc = get_nc()
    in_maps = prep_inputs(
        np.asarray(x, np.float32), np.asarray(w1, np.float32),
        np.asarray(b1, np.float32), np.asarray(w2, np.float32),
        np.asarray(b2, np.float32), np.asarray(w3, np.float32),
        np.asarray(b3, np.float32), np.asarray(mu, np.float32),
        np.asarray(label, np.float32))
    res = run_bass_kernel_spmd(nc, in_maps, core_ids=list(range(NCORES)),
                               **run_kwargs)
    out = gather(res.results)
    if run_kwargs:
        _cached['last_result'] = res
    return out


# revision 7
# speedup vs baseline: 1.4119x; 1.0078x over previous
"""AttentionClustering (vq_codebook) Trainium2 kernel, 8-core data parallel.

Shard: 8 cores = 4 images x 2 half-images (128 output rows each). Odd cores
get a vertically flipped shard + row-flipped conv weights so every core's
program is identical (true image edge at local top, interior halo at bottom).

Math: q1 = relu(conv3x3(x, w1) + b1); q2 = relu(conv3x3(q1, w2) + b2)  (both
with replicate padding); then the 1x1 conv + cluster-distance softmax folds to
  logit[px, k] = sum_ci q2[ci, px] * muW[k, ci] + bp[k]
  muW = 2 * mu @ W3,  bp = 2 * mu @ b3 - |mu|^2      (|q|^2 cancels in softmax)
  out[px] = sum_k softmax_k(logit) * label[k]

All matmuls run as float32r (12-bit mantissa, full PE rate at N>=256).
"""
import sys
if '/opt/trn_rl_repo' not in sys.path:
    sys.path.insert(0, '/opt/trn_rl_repo')

import numpy as np
import concourse.bass as bass
import concourse.mybir as mybir
from concourse import bacc, tile
from concourse.bass_utils import run_bass_kernel_spmd

F32 = mybir.dt.float32
F32R = mybir.dt.float32r
F16 = mybir.dt.float16
AF = mybir.ActivationFunctionType
ALU = mybir.AluOpType
AX = mybir.AxisListType

B, CIN, H, W = 4, 64, 256, 256
Q, K = 256, 16
RB = 32           # output rows per band
NBAND = 4         # bands per core (128 rows)
NCORES = 8

_cached = {}


def round_fp32r(x):
    u = np.ascontiguousarray(x, np.float32).view(np.uint32)
    lsb = (u >> 12) & 1
    return ((u + 0x7FF + lsb) & 0xFFFFF000).astype(np.uint32).view(np.float32)


def build_nc():
    nc = bacc.Bacc("TRN2", target_bir_lowering=False, debug=False)

    xh = nc.declare_dram_parameter("xh", [CIN, 132, W + 2], F16, isOutput=False)
    w1a = nc.declare_dram_parameter("w1a", [2, 3, 128, 128], F16, isOutput=False)
    w1r = nc.declare_dram_parameter("w1r", [2, 128, 128], F16, isOutput=False)
    w1s = nc.declare_dram_parameter("w1s", [2, 64, 128], F16, isOutput=False)
    w2l = nc.declare_dram_parameter("w2l", [36, 128, 128], F16, isOutput=False)
    muw = nc.declare_dram_parameter("muw", [2, 128, K], F16, isOutput=False)
    bp = nc.declare_dram_parameter("bp", [128, K], F32, isOutput=False)
    lab = nc.declare_dram_parameter("lab", [128, K], F32, isOutput=False)
    b1 = nc.declare_dram_parameter("b1", [2, 128, 1], F32, isOutput=False)
    b2 = nc.declare_dram_parameter("b2", [2, 128, 1], F32, isOutput=False)
    outd = nc.declare_dram_parameter("out", [128, W], F32, isOutput=True)

    with tile.TileContext(nc) as tc:
        with tc.tile_pool(name="singles", bufs=1) as singles, \
             tc.tile_pool(name="xpool", bufs=2) as xpool, \
             tc.tile_pool(name="q1pool", bufs=1) as q1pool, \
             tc.tile_pool(name="q2pool", bufs=2) as q2pool, \
             tc.tile_pool(name="smx", bufs=2) as smx, \
             tc.tile_pool(name="obuf", bufs=2) as obuf, \
             tc.tile_pool(name="ps1", bufs=3, space="PSUM") as ps1, \
             tc.tile_pool(name="ps2", bufs=2, space="PSUM") as ps2, \
             tc.tile_pool(name="psl", bufs=2, space="PSUM") as psl:

            # ---- resident weights -------------------------------------
            w1a_sb = {}
            for mc in range(2):
                for dr in range(3):
                    t = singles.tile([128, 128], F16, tag=f"w1a{mc}{dr}", name=f"w1a{mc}{dr}")
                    nc.sync.dma_start(out=t, in_=w1a.ap()[mc, dr])
                    w1a_sb[mc, dr] = t
            w1r_sb = {}
            w1s_sb = {}
            for mc in range(2):
                t = singles.tile([128, 128], F16, tag=f"w1r{mc}", name=f"w1r{mc}")
                nc.sync.dma_start(out=t, in_=w1r.ap()[mc])
                w1r_sb[mc] = t
                t = singles.tile([64, 128], F16, tag=f"w1s{mc}", name=f"w1s{mc}")
                nc.sync.dma_start(out=t, in_=w1s.ap()[mc])
                w1s_sb[mc] = t
            # band-0 x halo first so conv1 can start before w2 finishes loading
            def load_xband(r0):
                xa = xpool.tile([128, RB + 4, W + 2], F16, tag="xa", name="xa")
                nc.sync.dma_start(out=xa[0:64], in_=xh.ap()[:, r0:r0 + RB + 4, :])
                nc.sync.dma_start(out=xa[64:128, :, 0:W + 1],
                                  in_=xh.ap()[:, r0:r0 + RB + 4, 1:W + 2])
                xr = xpool.tile([128, RB + 4, W + 2], F16, tag="xr", name="xr")
                nc.sync.dma_start(out=xr[0:64], in_=xh.ap()[:, r0:r0 + RB + 4, :])
                nc.sync.dma_start(out=xr[64:128, 0:RB + 3, :],
                                  in_=xh.ap()[:, r0 + 1:r0 + RB + 4, :])
                return xa, xr

            xband0 = load_xband(0)

            # small constants next (before the bulky w2 tiles hog the queues)
            muw_sb = {}
            for kc in range(2):
                t = singles.tile([128, K], F16, tag=f"muw{kc}", name=f"muw{kc}")
                nc.sync.dma_start(out=t, in_=muw.ap()[kc])
                muw_sb[kc] = t
            bp_sb = singles.tile([128, K], F32, tag="bp")
            nc.sync.dma_start(out=bp_sb, in_=bp.ap())
            lab_sb = singles.tile([128, K], F32, tag="lab")
            nc.sync.dma_start(out=lab_sb, in_=lab.ap())
            b1_sb = {}
            b2_sb = {}
            for mc in range(2):
                t = singles.tile([128, 1], F32, tag=f"b1{mc}", name=f"b1{mc}")
                nc.sync.dma_start(out=t, in_=b1.ap()[mc])
                b1_sb[mc] = t
                t = singles.tile([128, 1], F32, tag=f"b2{mc}", name=f"b2{mc}")
                nc.sync.dma_start(out=t, in_=b2.ap()[mc])
                b2_sb[mc] = t

            # PE warmup: keep TensorE busy through the initial DMA wait so
            # the HAM clock-gate is at 8/8 when real matmuls arrive.
            wscr = singles.tile([128, 512], F16, tag="wscr")
            nc.vector.memset(wscr, 0.0)
            with tc.tile_pool(name="psw", bufs=1, space="PSUM") as psw:
                wps = psw.tile([128, 512], F32, tag="wps", name="wps")
                for _ in range(90):
                    nc.tensor.matmul(wps, wscr[:, 0:128], wscr,
                                     start=True, stop=True)

            w2_sb = {}
            for kc in range(2):
                for ti in range(9):
                    for mc in range(2):
                        idx = (kc * 9 + ti) * 2 + mc
                        t = singles.tile([128, 128], F16, tag=f"w2_{idx}", name=f"w2_{idx}")
                        nc.sync.dma_start(out=t, in_=w2l.ap()[idx])
                        w2_sb[kc, ti, mc] = t

            # ---- bands ------------------------------------------------
            for band in range(NBAND):
                r0 = RB * band
                # x halo in two packings:
                #  xa: p0-63 = xh rows r0..r0+19, p64-127 = same shifted +1 col
                #  xr: p0-63 = xh rows,           p64-127 = same shifted +1 row
                xa, xr = xband0 if band == 0 else load_xband(r0)

                # q1 band buffer: slot j = q1 row (r0 - 1 + j), cols 1..256
                # real, cols 0/257 replicate pads.
                q1b = {}
                for mc in range(2):
                    q1b[mc] = q1pool.tile([128, RB + 2, W + 2], F16, tag=f"q1_{mc}", name=f"q1_{mc}")

                # conv1: q1 slot j needs xh local rows j+dr (pairs), and
                # taps (0,2),(1,2) from xr row j, tap (2,2) from xa row j+2.
                if band == 0:
                    groups1 = [(j, 2) for j in range(1, RB + 1, 2)] + [(RB + 1, 1)]
                else:
                    groups1 = [(j, 2) for j in range(0, RB + 2, 2)]
                for j, nr in groups1:
                    for mc in range(2):
                        ps = ps1.tile([128, nr, W], F32, tag="c1ps", name="c1ps")
                        for dr in range(3):
                            nc.tensor.matmul(
                                ps, w1a_sb[mc, dr],
                                xa[:, j + dr:j + dr + nr, 0:W],
                                start=(dr == 0), stop=False)
                        nc.tensor.matmul(ps, w1r_sb[mc],
                                         xr[:, j:j + nr, 2:W + 2],
                                         start=False, stop=False)
                        nc.tensor.matmul(ps, w1s_sb[mc],
                                         xa[0:64, j + 2:j + 2 + nr, 2:W + 2],
                                         start=False, stop=True)
                        nc.scalar.activation(
                            out=q1b[mc][:, j:j + nr, 1:W + 1], in_=ps,
                            func=AF.Relu, bias=b1_sb[mc], scale=1.0)
                # replicate pads: cols, then (band 0) top row
                for mc in range(2):
                    lo = 1 if band == 0 else 0
                    nc.vector.tensor_copy(
                        out=q1b[mc][:, lo:RB + 2, 0:1],
                        in_=q1b[mc][:, lo:RB + 2, 1:2])
                    nc.vector.tensor_copy(
                        out=q1b[mc][:, lo:RB + 2, W + 1:W + 2],
                        in_=q1b[mc][:, lo:RB + 2, W:W + 1])
                    if band == 0:
                        nc.vector.tensor_copy(
                            out=q1b[mc][:, 0:1, :], in_=q1b[mc][:, 1:2, :])

                ob = obuf.tile([128, RB // 2, 4], F32, tag="ob", name="ob")
                for g in range(RB // 2):
                    # conv2 -> q2 (2 output rows x 256 cols per group)
                    q2t = {}
                    for mc in range(2):
                        ps = ps2.tile([128, 2, W], F32, tag="c2ps", name="c2ps")
                        n_mm = 0
                        for kc in range(2):
                            for dr in range(3):
                                for dc in range(3):
                                    nc.tensor.matmul(
                                        ps, w2_sb[kc, dr * 3 + dc, mc],
                                        q1b[kc][:, 2 * g + dr:2 * g + dr + 2,
                                                dc:dc + W],
                                        start=(n_mm == 0), stop=(n_mm == 17))
                                    n_mm += 1
                        q2t[mc] = q2pool.tile([128, 2, W], F16, tag=f"q2_{mc}", name=f"q2_{mc}")
                        nc.scalar.activation(out=q2t[mc], in_=ps, func=AF.Relu,
                                             bias=b2_sb[mc], scale=1.0)
                    # logits: [128 px, K] per 128-px slice, q2 stationary
                    pl = psl.tile([128, 4, K], F32, tag="lps", name="lps")
                    for j in range(4):
                        for kc in range(2):
                            q2flat = q2t[kc].rearrange("p a b -> p (a b)")
                            nc.tensor.matmul(
                                pl[:, j, :], q2flat[:, 128 * j:128 * (j + 1)],
                                muw_sb[kc], start=(kc == 0), stop=(kc == 1))
                    # softmax over K (free axis) + label contraction
                    li = smx.tile([128, 4, K], F32, tag="li", name="li")
                    nc.vector.tensor_tensor(
                        li, pl,
                        bp_sb.unsqueeze(1).to_broadcast([128, 4, K]),
                        ALU.add)
                    mx = smx.tile([128, 4], F32, tag="mx", name="mx")
                    nc.vector.reduce_max(mx, li, axis=AX.X)
                    ls = smx.tile([128, 4, K], F32, tag="ls", name="ls")
                    nc.vector.tensor_tensor(
                        ls, li,
                        mx.unsqueeze(2).to_broadcast([128, 4, K]),
                        ALU.subtract)
                    ex = smx.tile([128, 4, K], F32, tag="ex", name="ex")
                    nc.scalar.activation(out=ex, in_=ls, func=AF.Exp)
                    el = smx.tile([128, 4, K], F32, tag="el", name="el")
                    nc.vector.tensor_tensor(
                        el, ex,
                        lab_sb.unsqueeze(1).to_broadcast([128, 4, K]),
                        ALU.mult)
                    ssum = smx.tile([128, 4], F32, tag="ssum", name="ssum")
                    nc.vector.reduce_sum(ssum, ex, axis=AX.X)
                    wsum = smx.tile([128, 4], F32, tag="wsum", name="wsum")
                    nc.vector.reduce_sum(wsum, el, axis=AX.X)
                    rs = smx.tile([128, 4], F32, tag="rs", name="rs")
                    nc.vector.reciprocal(rs, ssum)
                    nc.vector.tensor_tensor(ob[:, g], wsum, rs, ALU.mult)

                # out[p, g, r, jj] -> dram row r0+2g+r, col 128*jj + p
                nc.sync.dma_start(
                    out=outd.ap()[r0:r0 + RB, :].rearrange(
                        "(g r) (jj p) -> p g r jj", r=2, p=128),
                    in_=ob.rearrange("p g (r jj) -> p g r jj", r=2))

    nc.compile()
    return nc


def prep_inputs(x, w1, b1, w2, b2, w3, b3, mu, label):
    """Full inputs -> per-core in_maps."""
    w3m = w3[:, :, 0, 0]
    muW = 2.0 * (mu @ w3m)                                   # [K, Q]
    bpv = (2.0 * (mu @ b3) - (mu * mu).sum(1)).astype(np.float32)

    def pack_w(w1f, w2f):
        cinw = w1f.shape[1]
        w1a = np.empty((2, 3, 128, 128), np.float32)
        w1r = np.empty((2, 128, 128), np.float32)
        w1s = np.empty((2, 64, 128), np.float32)
        for mc in range(2):
            ms = slice(128 * mc, 128 * (mc + 1))
            for dr in range(3):
                w1a[mc, dr, 0:64] = w1f[ms, :, dr, 0].T
                w1a[mc, dr, 64:128] = w1f[ms, :, dr, 1].T
            w1r[mc, 0:64] = w1f[ms, :, 0, 2].T
            w1r[mc, 64:128] = w1f[ms, :, 1, 2].T
            w1s[mc] = w1f[ms, :, 2, 2].T
        w2p = np.empty((36, 128, 128), np.float32)
        for kc in range(2):
            for dr in range(3):
                for dc in range(3):
                    for mc in range(2):
                        idx = (kc * 9 + dr * 3 + dc) * 2 + mc
                        w2p[idx] = w2f[128 * mc:128 * (mc + 1),
                                       128 * kc:128 * (kc + 1), dr, dc].T
        return (w1a.astype(np.float16), w1r.astype(np.float16),
                w1s.astype(np.float16), w2p.astype(np.float16))

    packs = {}
    packs[0] = pack_w(w1, w2)
    packs[1] = pack_w(w1[:, :, ::-1, :], w2[:, :, ::-1, :])

    muwp = np.empty((2, 128, K), np.float32)
    for kc in range(2):
        muwp[kc] = muW[:, 128 * kc:128 * (kc + 1)].T
    muwp = muwp.astype(np.float16)
    bpt = np.broadcast_to(bpv[None, :], (128, K)).copy()
    labt = np.broadcast_to(label[None, :].astype(np.float32), (128, K)).copy()
    b1t = np.empty((2, 128, 1), np.float32)
    b2t = np.empty((2, 128, 1), np.float32)
    for mc in range(2):
        b1t[mc, :, 0] = b1[128 * mc:128 * (mc + 1)]
        b2t[mc, :, 0] = b2[128 * mc:128 * (mc + 1)]

    rows = np.clip(np.arange(132) - 2, 0, H - 1)
    cols = np.clip(np.arange(W + 2) - 1, 0, W - 1)
    in_maps = []
    for core in range(NCORES):
        img, half = core // 2, core % 2
        xl = x[img] if half == 0 else x[img, :, ::-1, :]
        xhv = np.ascontiguousarray(xl[:, rows][:, :, cols]).astype(np.float16)
        w1ap, w1rp, w1sp, w2p = packs[half]
        in_maps.append({
            'xh': xhv, 'w1a': w1ap, 'w1r': w1rp, 'w1s': w1sp, 'w2l': w2p,
            'muw': muwp, 'bp': bpt, 'lab': labt, 'b1': b1t, 'b2': b2t,
        })
    return in_maps


def gather(results, dtype=np.float32):
    out = np.empty((B, 1, H, W), dtype)
    for core in range(NCORES):
        img, half = core // 2, core % 2
        o = results[core]['out']
        if half == 0:
            out[img, 0, 0:128] = o
        else:
            out[img, 0, 128:256] = o[::-1]
    return out


def get_nc():
    if 'nc' not in _cached:
        _cached['nc'] = build_nc()
    return _cached['nc']


def kernel(x, w1, b1, w2, b2, w3, b3, mu, label, **run_kwargs):
    nc = get_nc()
    in_maps = prep_inputs(
        np.asarray(x, np.float32), np.asarray(w1, np.float32),
        np.asarray(b1, np.float32), np.asarray(w2, np.float32),
        np.asarray(b2, np.float32), np.asarray(w3, np.float32),
        np.asarray(b3, np.float32), np.asarray(mu, np.float32),
        np.asarray(label, np.float32))
    res = run_bass_kernel_spmd(nc, in_maps, core_ids=list(range(NCORES)),
                               **run_kwargs)
    out = gather(res.results)
    if run_kwargs:
        _cached['last_result'] = res
    return out


# revision 8
# speedup vs baseline: 1.4336x; 1.0153x over previous
"""AttentionClustering (vq_codebook) Trainium2 kernel, 8-core data parallel.

Shard: 8 cores = 4 images x 2 half-images (128 output rows each). Odd cores
get a vertically flipped shard + row-flipped conv weights so every core's
program is identical (true image edge at local top, interior halo at bottom).

Math: q1 = relu(conv3x3(x, w1) + b1); q2 = relu(conv3x3(q1, w2) + b2)  (both
with replicate padding); then the 1x1 conv + cluster-distance softmax folds to
  logit[px, k] = sum_ci q2[ci, px] * muW[k, ci] + bp[k]
  muW = 2 * mu @ W3,  bp = 2 * mu @ b3 - |mu|^2      (|q|^2 cancels in softmax)
  out[px] = sum_k softmax_k(logit) * label[k]

All matmuls run as float32r (12-bit mantissa, full PE rate at N>=256).
"""
import sys
if '/opt/trn_rl_repo' not in sys.path:
    sys.path.insert(0, '/opt/trn_rl_repo')

import numpy as np
import concourse.bass as bass
import concourse.mybir as mybir
from concourse import bacc, tile
from concourse.bass_utils import run_bass_kernel_spmd

F32 = mybir.dt.float32
F32R = mybir.dt.float32r
F16 = mybir.dt.float16
AF = mybir.ActivationFunctionType
ALU = mybir.AluOpType
AX = mybir.AxisListType

B, CIN, H, W = 4, 64, 256, 256
Q, K = 256, 16
RB = 32           # output rows per band
NBAND = 4         # bands per core (128 rows)
NCORES = 8

_cached = {}


def round_fp32r(x):
    u = np.ascontiguousarray(x, np.float32).view(np.uint32)
    lsb = (u >> 12) & 1
    return ((u + 0x7FF + lsb) & 0xFFFFF000).astype(np.uint32).view(np.float32)


def build_nc():
    nc = bacc.Bacc("TRN2", target_bir_lowering=False, debug=False)

    xh = nc.declare_dram_parameter("xh", [CIN, 132, W + 2], F16, isOutput=False)
    w1a = nc.declare_dram_parameter("w1a", [2, 3, 128, 128], F16, isOutput=False)
    w1r = nc.declare_dram_parameter("w1r", [2, 128, 128], F16, isOutput=False)
    w1s = nc.declare_dram_parameter("w1s", [2, 64, 128], F16, isOutput=False)
    w2l = nc.declare_dram_parameter("w2l", [36, 128, 128], F16, isOutput=False)
    muw = nc.declare_dram_parameter("muw", [2, 128, K], F16, isOutput=False)
    bp = nc.declare_dram_parameter("bp", [128, K], F32, isOutput=False)
    lab = nc.declare_dram_parameter("lab", [128, K], F32, isOutput=False)
    b1 = nc.declare_dram_parameter("b1", [2, 128, 1], F32, isOutput=False)
    b2 = nc.declare_dram_parameter("b2", [2, 128, 1], F32, isOutput=False)
    outd = nc.declare_dram_parameter("out", [128, W], F32, isOutput=True)

    with tile.TileContext(nc) as tc:
        with tc.tile_pool(name="singles", bufs=1) as singles, \
             tc.tile_pool(name="xpool", bufs=2) as xpool, \
             tc.tile_pool(name="q1pool", bufs=1) as q1pool, \
             tc.tile_pool(name="q2pool", bufs=2) as q2pool, \
             tc.tile_pool(name="smx", bufs=2) as smx, \
             tc.tile_pool(name="obuf", bufs=2) as obuf, \
             tc.tile_pool(name="ps1", bufs=3, space="PSUM") as ps1, \
             tc.tile_pool(name="ps2", bufs=2, space="PSUM") as ps2, \
             tc.tile_pool(name="psl", bufs=2, space="PSUM") as psl:

            # ---- resident weights -------------------------------------
            w1a_sb = {}
            for mc in range(2):
                for dr in range(3):
                    t = singles.tile([128, 128], F16, tag=f"w1a{mc}{dr}", name=f"w1a{mc}{dr}")
                    nc.sync.dma_start(out=t, in_=w1a.ap()[mc, dr])
                    w1a_sb[mc, dr] = t
            w1r_sb = {}
            w1s_sb = {}
            for mc in range(2):
                t = singles.tile([128, 128], F16, tag=f"w1r{mc}", name=f"w1r{mc}")
                nc.sync.dma_start(out=t, in_=w1r.ap()[mc])
                w1r_sb[mc] = t
                t = singles.tile([64, 128], F16, tag=f"w1s{mc}", name=f"w1s{mc}")
                nc.sync.dma_start(out=t, in_=w1s.ap()[mc])
                w1s_sb[mc] = t
            # band-0 x halo first so conv1 can start before w2 finishes loading
            def load_xband(r0, split=False):
                # split=True: two row-chunks per buffer so band-0 conv1 can
                # start as soon as the first rows land.
                chunks = [(0, 20), (20, RB + 4)] if split else [(0, RB + 4)]
                xa = xpool.tile([128, RB + 4, W + 2], F16, tag="xa", name="xa")
                xr = xpool.tile([128, RB + 4, W + 2], F16, tag="xr", name="xr")
                for lo, hi in chunks:
                    nc.sync.dma_start(out=xa[0:64, lo:hi, :],
                                      in_=xh.ap()[:, r0 + lo:r0 + hi, :])
                    nc.sync.dma_start(out=xa[64:128, lo:hi, 0:W + 1],
                                      in_=xh.ap()[:, r0 + lo:r0 + hi, 1:W + 2])
                    nc.sync.dma_start(out=xr[0:64, lo:hi, :],
                                      in_=xh.ap()[:, r0 + lo:r0 + hi, :])
                    hi2 = min(hi, RB + 3)
                    nc.sync.dma_start(out=xr[64:128, lo:hi2, :],
                                      in_=xh.ap()[:, r0 + 1 + lo:r0 + 1 + hi2, :])
                return xa, xr

            xband0 = load_xband(0, split=True)

            # small constants next (before the bulky w2 tiles hog the queues)
            muw_sb = {}
            for kc in range(2):
                t = singles.tile([128, K], F16, tag=f"muw{kc}", name=f"muw{kc}")
                nc.sync.dma_start(out=t, in_=muw.ap()[kc])
                muw_sb[kc] = t
            bp_sb = singles.tile([128, K], F32, tag="bp")
            nc.sync.dma_start(out=bp_sb, in_=bp.ap())
            lab_sb = singles.tile([128, K], F32, tag="lab")
            nc.sync.dma_start(out=lab_sb, in_=lab.ap())
            b1_sb = {}
            b2_sb = {}
            for mc in range(2):
                t = singles.tile([128, 1], F32, tag=f"b1{mc}", name=f"b1{mc}")
                nc.sync.dma_start(out=t, in_=b1.ap()[mc])
                b1_sb[mc] = t
                t = singles.tile([128, 1], F32, tag=f"b2{mc}", name=f"b2{mc}")
                nc.sync.dma_start(out=t, in_=b2.ap()[mc])
                b2_sb[mc] = t

            # PE warmup: keep TensorE busy through the initial DMA wait so
            # the HAM clock-gate is at 8/8 when real matmuls arrive.
            wscr = singles.tile([128, 512], F16, tag="wscr")
            nc.vector.memset(wscr, 0.0)
            with tc.tile_pool(name="psw", bufs=1, space="PSUM") as psw:
                wps = psw.tile([128, 512], F32, tag="wps", name="wps")
                for _ in range(40):
                    nc.tensor.matmul(wps, wscr[:, 0:128], wscr,
                                     start=True, stop=True)

            w2_sb = {}
            for kc in range(2):
                for ti in range(9):
                    for mc in range(2):
                        idx = (kc * 9 + ti) * 2 + mc
                        t = singles.tile([128, 128], F16, tag=f"w2_{idx}", name=f"w2_{idx}")
                        nc.sync.dma_start(out=t, in_=w2l.ap()[idx])
                        w2_sb[kc, ti, mc] = t

            # ---- bands ------------------------------------------------
            for band in range(NBAND):
                r0 = RB * band
                # x halo in two packings:
                #  xa: p0-63 = xh rows r0..r0+19, p64-127 = same shifted +1 col
                #  xr: p0-63 = xh rows,           p64-127 = same shifted +1 row
                xa, xr = xband0 if band == 0 else load_xband(r0)

                # q1 band buffer: slot j = q1 row (r0 - 1 + j), cols 1..256
                # real, cols 0/257 replicate pads.
                q1b = {}
                for mc in range(2):
                    q1b[mc] = q1pool.tile([128, RB + 2, W + 2], F16, tag=f"q1_{mc}", name=f"q1_{mc}")

                # conv1: q1 slot j needs xh local rows j+dr (pairs), and
                # taps (0,2),(1,2) from xr row j, tap (2,2) from xa row j+2.
                if band == 0:
                    groups1 = [(j, 2) for j in range(1, RB + 1, 2)] + [(RB + 1, 1)]
                else:
                    groups1 = [(j, 2) for j in range(0, RB + 2, 2)]
                for j, nr in groups1:
                    for mc in range(2):
                        ps = ps1.tile([128, nr, W], F32, tag="c1ps", name="c1ps")
                        for dr in range(3):
                            nc.tensor.matmul(
                                ps, w1a_sb[mc, dr],
                                xa[:, j + dr:j + dr + nr, 0:W],
                                start=(dr == 0), stop=False)
                        nc.tensor.matmul(ps, w1r_sb[mc],
                                         xr[:, j:j + nr, 2:W + 2],
                                         start=False, stop=False)
                        nc.tensor.matmul(ps, w1s_sb[mc],
                                         xa[0:64, j + 2:j + 2 + nr, 2:W + 2],
                                         start=False, stop=True)
                        nc.scalar.activation(
                            out=q1b[mc][:, j:j + nr, 1:W + 1], in_=ps,
                            func=AF.Relu, bias=b1_sb[mc], scale=1.0)
                # replicate pads: cols, then (band 0) top row
                for mc in range(2):
                    lo = 1 if band == 0 else 0
                    nc.vector.tensor_copy(
                        out=q1b[mc][:, lo:RB + 2, 0:1],
                        in_=q1b[mc][:, lo:RB + 2, 1:2])
                    nc.vector.tensor_copy(
                        out=q1b[mc][:, lo:RB + 2, W + 1:W + 2],
                        in_=q1b[mc][:, lo:RB + 2, W:W + 1])
                    if band == 0:
                        nc.vector.tensor_copy(
                            out=q1b[mc][:, 0:1, :], in_=q1b[mc][:, 1:2, :])

                ob = obuf.tile([128, RB // 2, 4], F32, tag="ob", name="ob")
                for g in range(RB // 2):
                    # conv2 -> q2 (2 output rows x 256 cols per group)
                    q2t = {}
                    for mc in range(2):
                        ps = ps2.tile([128, 2, W], F32, tag="c2ps", name="c2ps")
                        n_mm = 0
                        for kc in range(2):
                            for dr in range(3):
                                for dc in range(3):
                                    nc.tensor.matmul(
                                        ps, w2_sb[kc, dr * 3 + dc, mc],
                                        q1b[kc][:, 2 * g + dr:2 * g + dr + 2,
                                                dc:dc + W],
                                        start=(n_mm == 0), stop=(n_mm == 17))
                                    n_mm += 1
                        q2t[mc] = q2pool.tile([128, 2, W], F16, tag=f"q2_{mc}", name=f"q2_{mc}")
                        nc.scalar.activation(out=q2t[mc], in_=ps, func=AF.Relu,
                                             bias=b2_sb[mc], scale=1.0)
                    # logits: [128 px, K] per 128-px slice, q2 stationary
                    pl = psl.tile([128, 4, K], F32, tag="lps", name="lps")
                    for j in range(4):
                        for kc in range(2):
                            q2flat = q2t[kc].rearrange("p a b -> p (a b)")
                            nc.tensor.matmul(
                                pl[:, j, :], q2flat[:, 128 * j:128 * (j + 1)],
                                muw_sb[kc], start=(kc == 0), stop=(kc == 1))
                    # softmax over K (free axis) + label contraction
                    li = smx.tile([128, 4, K], F32, tag="li", name="li")
                    nc.vector.tensor_tensor(
                        li, pl,
                        bp_sb.unsqueeze(1).to_broadcast([128, 4, K]),
                        ALU.add)
                    mx = smx.tile([128, 4], F32, tag="mx", name="mx")
                    nc.vector.reduce_max(mx, li, axis=AX.X)
                    ls = smx.tile([128, 4, K], F32, tag="ls", name="ls")
                    nc.vector.tensor_tensor(
                        ls, li,
                        mx.unsqueeze(2).to_broadcast([128, 4, K]),
                        ALU.subtract)
                    ex = smx.tile([128, 4, K], F32, tag="ex", name="ex")
                    nc.scalar.activation(out=ex, in_=ls, func=AF.Exp)
                    el = smx.tile([128, 4, K], F32, tag="el", name="el")
                    nc.vector.tensor_tensor(
                        el, ex,
                        lab_sb.unsqueeze(1).to_broadcast([128, 4, K]),
                        ALU.mult)
                    ssum = smx.tile([128, 4], F32, tag="ssum", name="ssum")
                    nc.vector.reduce_sum(ssum, ex, axis=AX.X)
                    wsum = smx.tile([128, 4], F32, tag="wsum", name="wsum")
                    nc.vector.reduce_sum(wsum, el, axis=AX.X)
                    rs = smx.tile([128, 4], F32, tag="rs", name="rs")
                    nc.vector.reciprocal(rs, ssum)
                    nc.vector.tensor_tensor(ob[:, g], wsum, rs, ALU.mult)

                # out[p, g, r, jj] -> dram row r0+2g+r, col 128*jj + p
                nc.sync.dma_start(
                    out=outd.ap()[r0:r0 + RB, :].rearrange(
                        "(g r) (jj p) -> p g r jj", r=2, p=128),
                    in_=ob.rearrange("p g (r jj) -> p g r jj", r=2))

    nc.compile()
    return nc


def prep_inputs(x, w1, b1, w2, b2, w3, b3, mu, label):
    """Full inputs -> per-core in_maps."""
    w3m = w3[:, :, 0, 0]
    muW = 2.0 * (mu @ w3m)                                   # [K, Q]
    bpv = (2.0 * (mu @ b3) - (mu * mu).sum(1)).astype(np.float32)

    def pack_w(w1f, w2f):
        cinw = w1f.shape[1]
        w1a = np.empty((2, 3, 128, 128), np.float32)
        w1r = np.empty((2, 128, 128), np.float32)
        w1s = np.empty((2, 64, 128), np.float32)
        for mc in range(2):
            ms = slice(128 * mc, 128 * (mc + 1))
            for dr in range(3):
                w1a[mc, dr, 0:64] = w1f[ms, :, dr, 0].T
                w1a[mc, dr, 64:128] = w1f[ms, :, dr, 1].T
            w1r[mc, 0:64] = w1f[ms, :, 0, 2].T
            w1r[mc, 64:128] = w1f[ms, :, 1, 2].T
            w1s[mc] = w1f[ms, :, 2, 2].T
        w2p = np.empty((36, 128, 128), np.float32)
        for kc in range(2):
            for dr in range(3):
                for dc in range(3):
                    for mc in range(2):
                        idx = (kc * 9 + dr * 3 + dc) * 2 + mc
                        w2p[idx] = w2f[128 * mc:128 * (mc + 1),
                                       128 * kc:128 * (kc + 1), dr, dc].T
        return (w1a.astype(np.float16), w1r.astype(np.float16),
                w1s.astype(np.float16), w2p.astype(np.float16))

    packs = {}
    packs[0] = pack_w(w1, w2)
    packs[1] = pack_w(w1[:, :, ::-1, :], w2[:, :, ::-1, :])

    muwp = np.empty((2, 128, K), np.float32)
    for kc in range(2):
        muwp[kc] = muW[:, 128 * kc:128 * (kc + 1)].T
    muwp = muwp.astype(np.float16)
    bpt = np.broadcast_to(bpv[None, :], (128, K)).copy()
    labt = np.broadcast_to(label[None, :].astype(np.float32), (128, K)).copy()
    b1t = np.empty((2, 128, 1), np.float32)
    b2t = np.empty((2, 128, 1), np.float32)
    for mc in range(2):
        b1t[mc, :, 0] = b1[128 * mc:128 * (mc + 1)]
        b2t[mc, :, 0] = b2[128 * mc:128 * (mc + 1)]

    rows = np.clip(np.arange(132) - 2, 0, H - 1)
    cols = np.clip(np.arange(W + 2) - 1, 0, W - 1)
    in_maps = []
    for core in range(NCORES):
        img, half = core // 2, core % 2
        xl = x[img] if half == 0 else x[img, :, ::-1, :]
        xhv = np.ascontiguousarray(xl[:, rows][:, :, cols]).astype(np.float16)
        w1ap, w1rp, w1sp, w2p = packs[half]
        in_maps.append({
            'xh': xhv, 'w1a': w1ap, 'w1r': w1rp, 'w1s': w1sp, 'w2l': w2p,
            'muw': muwp, 'bp': bpt, 'lab': labt, 'b1': b1t, 'b2': b2t,
        })
    return in_maps


def gather(results, dtype=np.float32):
    out = np.empty((B, 1, H, W), dtype)
    for core in range(NCORES):
        img, half = core // 2, core % 2
        o = results[core]['out']
        if half == 0:
            out[img, 0, 0:128] = o
        else:
            out[img, 0, 128:256] = o[::-1]
    return out


def get_nc():
    if 'nc' not in _cached:
        _cached['nc'] = build_nc()
    return _cached['nc']


def kernel(x, w1, b1, w2, b2, w3, b3, mu, label, **run_kwargs):
    nc = get_nc()
    in_maps = prep_inputs(
        np.asarray(x, np.float32), np.asarray(w1, np.float32),
        np.asarray(b1, np.float32), np.asarray(w2, np.float32),
        np.asarray(b2, np.float32), np.asarray(w3, np.float32),
        np.asarray(b3, np.float32), np.asarray(mu, np.float32),
        np.asarray(label, np.float32))
    res = run_bass_kernel_spmd(nc, in_maps, core_ids=list(range(NCORES)),
                               **run_kwargs)
    out = gather(res.results)
    if run_kwargs:
        _cached['last_result'] = res
    return out
